# revision 3
# baseline (speedup 1.0000x reference)
"""GCN (3-layer, PyG-style) forward on 8 Trainium2 NeuronCores.

Math restructuring
------------------
reference:
  h1 = relu(Anorm @ x @ W1 + b1)          (Anorm includes self loops + sym norm)
  h2 = relu(Anorm @ h1 @ W2 + b2)
  h3 = Anorm @ h2 @ W3 + b3
  out = segment_mean(h3, batch) @ Wlin + blin

Because GCNConv aggregation and the weight matmul commute, and pooling is
linear, this is equivalent to:
  agg1 = Anorm @ x                        # [N,2]  (tiny -> host)
  h1   = relu(agg1 @ W1 + b1)             # rank-3 structure: per-EDGE on device
  g2   = Anorm @ h1                       # one-hot scatter matmul on device
  h2   = relu(g2 @ W2 + b2)               # dense matmul on device
  pg3[g] = sum_e norm_e * h2[row_e] * [batch[col_e]==g]   # dense T.T@h2 on device
  out  = ((pg3 @ W3 + cnt*b3)/max(cnt,1)) @ Wlin + blin   # [128,1024] -> host

Sharding: nodes (and L2 edges by dst / L3 edges by src) are partitioned into
8 contiguous blocks of 12500.  Every core runs the same program (SPMD) on its
own edge arrays, padded to identical tile counts.  Per-core output is a
partial pg3 [128,1024]; the host sums them (the "all-reduce").
"""

import numpy as np

N_NODES = 100000
N_EDGES = 400000
G = 128
FIN = 2
H = 1024
N_CORES = 8
NPC = N_NODES // N_CORES          # 12500 nodes per core
P = 128
NW = (NPC + P - 1) // P           # 98 windows (last has 84 dsts)


def _host_prep(x, edge_index, batch):
    """All O(E) index work in numpy; returns per-core device arrays."""
    x = np.asarray(x, dtype=np.float32)
    ei = np.asarray(edge_index).astype(np.int64)
    batch = np.asarray(batch).astype(np.int64)
    n = N_NODES

    loops = np.arange(n, dtype=np.int64)
    row = np.concatenate([ei[0], loops])
    col = np.concatenate([ei[1], loops])

    deg = np.bincount(col, minlength=n).astype(np.float64)
    dis = np.where(deg > 0, 1.0 / np.sqrt(np.maximum(deg, 1.0)), 0.0)
    norm = (dis[row] * dis[col]).astype(np.float64)

    # layer-1 aggregation (FIN=2) on host
    agg1 = np.empty((n, FIN), dtype=np.float32)
    for f in range(FIN):
        agg1[:, f] = np.bincount(
            col, weights=norm * x[row, f].astype(np.float64), minlength=n
        ).astype(np.float32)

    norm = norm.astype(np.float32)

    # ---- L2 edge partition by destination core, sorted by col ----
    core_of = col // NPC
    order = np.argsort(col, kind="stable")
    row_s, col_s, norm_s = row[order], col[order], norm[order]
    core_s = core_of[order]

    # window of each edge (within its core), and counts per (core, window)
    col_local = col_s - core_s * NPC
    win = col_local // P                       # 0..NW-1
    cw = core_s * NW + win
    cw_counts = np.bincount(cw, minlength=N_CORES * NW).reshape(N_CORES, NW)
    tiles_per_cw = (cw_counts + P - 1) // P
    T_w = tiles_per_cw.max(axis=0)             # shared tile count per window
    base_tile = np.concatenate([[0], np.cumsum(T_w)])
    TT = int(base_tile[-1])                    # total edge tiles per core

    # position of each edge within its (core, window) run
    cw_starts = np.concatenate([[0], np.cumsum(cw_counts.reshape(-1))])
    idx_in_cw = np.arange(len(col_s)) - cw_starts[cw]
    tile_in_w = idx_in_cw // P
    pos = idx_in_cw % P
    tile_global = base_tile[win] + tile_in_w   # 0..TT-1 within the core

    # per-core device arrays
    aT = np.zeros((N_CORES, 4, TT * P), dtype=np.float32)
    S = np.zeros((N_CORES, TT * P, P), dtype=np.float32)
    slot = tile_global * P + pos
    dst_in_win = col_local - win * P
    c = core_s
    aT[c, 0, slot] = agg1[row_s, 0]
    aT[c, 1, slot] = agg1[row_s, 1]
    aT[c, 2, slot] = 1.0
    S[c, slot, dst_in_win] = norm_s

    # ---- L3: T matrix, edges partitioned by source ----
    gcol = batch[col]                          # graph of each edge's dst
    Tmat = np.bincount(
        row * G + gcol, weights=norm.astype(np.float64), minlength=n * G
    ).astype(np.float32).reshape(n, G)
    # reshape to per-core [NW*P, G] with zero padding rows
    Tpad = np.zeros((N_CORES, NW * P, G), dtype=np.float32)
    Tpad[:, :NPC, :] = Tmat.reshape(N_CORES, NPC, G)

    cnt = np.bincount(batch, minlength=G).astype(np.float32)
    return agg1, aT, S, Tpad, cnt, T_w, TT


def _build_device_program(TT, T_w, nw=NW):
    import concourse.mybir as mybir
    import concourse.tile as tile
    from concourse import bacc
    from concourse.masks import make_identity

    f32 = mybir.dt.float32
    nc = bacc.Bacc(None, target_bir_lowering=False, debug=False)

    aT_d = nc.dram_tensor("aT", [4, TT * P], f32, kind="ExternalInput")
    S_d = nc.dram_tensor("S", [TT, P, P], f32, kind="ExternalInput")
    T_d = nc.dram_tensor("T", [NW, P, G], f32, kind="ExternalInput")
    W1b_d = nc.dram_tensor("W1b", [4, H], f32, kind="ExternalInput")
    W2_d = nc.dram_tensor("W2", [8, P, H], f32, kind="ExternalInput")
    b2_d = nc.dram_tensor("b2", [1, H], f32, kind="ExternalInput")
    out_d = nc.dram_tensor("pg3", [G, H], f32, kind="ExternalOutput")

    CH = 32  # aT tiles per staged chunk

    with tile.TileContext(nc) as tc:
        with (
            tc.tile_pool(name="const", bufs=1) as cst,
            tc.tile_pool(name="sa", bufs=2) as sa,
            tc.tile_pool(name="sS", bufs=4) as sS,
            tc.tile_pool(name="smsg", bufs=3) as smsg,
            tc.tile_pool(name="sg2s", bufs=2) as sg2s,
            tc.tile_pool(name="sg2T", bufs=2) as sg2T,
            tc.tile_pool(name="sh2", bufs=2) as sh2,
            tc.tile_pool(name="sT", bufs=2) as sT,
            tc.tile_pool(name="zp", bufs=3, space="PSUM") as zp,
            tc.tile_pool(name="g2p", bufs=1, space="PSUM") as g2p,
            tc.tile_pool(name="hp", bufs=3, space="PSUM") as hp,
        ):
            Relu = mybir.ActivationFunctionType.Relu
            Copy = mybir.ActivationFunctionType.Copy

            W1b = cst.tile([4, H], f32, tag="W1b")
            nc.sync.dma_start(W1b[:], W1b_d[:])
            W2s = cst.tile([P, 8, H], f32, tag="W2s")
            nc.sync.dma_start(W2s[:], W2_d[:].rearrange("c p f -> p c f"))
            b2s = cst.tile([1, H], f32, tag="b2s")
            nc.sync.dma_start(b2s[:], b2_d[:])
            ones1 = cst.tile([1, P], f32, tag="ones1")
            nc.vector.memset(ones1[:], 1.0)
            ident = cst.tile([P, P], f32, tag="ident")
            make_identity(nc, ident[:])
            pg3s = cst.tile([G, H], f32, tag="pg3s")
            nc.vector.memset(pg3s[:], 0.0)

            aTc = None
            for w in range(nw):
                Tt = sT.tile([P, G], f32, tag="Tt")
                nc.sync.dma_start(Tt[:], T_d[w])

                g2 = g2p.tile([P, H], f32, tag="g2")
                nt = int(T_w[w])
                for t in range(nt):
                    tg = int(np.sum(T_w[:w])) + t
                    if tg % CH == 0:
                        aTc = sa.tile([4, CH * P], f32, tag="aTc")
                        hi = min((tg + CH) * P, TT * P)
                        nc.sync.dma_start(
                            aTc[:, : hi - tg * P], aT_d[:, tg * P : hi]
                        )
                    off = (tg % CH) * P
                    Ss = sS.tile([P, P], f32, tag="Ss")
                    nc.sync.dma_start(Ss[:], S_d[tg])

                    zA = zp.tile([P, 512], f32, tag="z")
                    zB = zp.tile([P, 512], f32, tag="z")
                    lhs_a = aTc[:, off : off + P]
                    nc.tensor.matmul(zA[:], lhs_a, W1b[:, :512], start=True, stop=True)
                    nc.tensor.matmul(zB[:], lhs_a, W1b[:, 512:], start=True, stop=True)
                    msg = smsg.tile([P, H], f32, tag="msg")
                    nc.scalar.activation(msg[:, :512], zA[:], Relu)
                    nc.vector.tensor_scalar_max(msg[:, 512:], zB[:], 0.0)

                    nc.tensor.matmul(
                        g2[:, :512], Ss[:], msg[:, :512],
                        start=(t == 0), stop=(t == nt - 1),
                    )
                    nc.tensor.matmul(
                        g2[:, 512:], Ss[:], msg[:, 512:],
                        start=(t == 0), stop=(t == nt - 1),
                    )

                # g2 [128 dst, 1024] -> transpose to g2T [1024 fin, 128 dst]
                g2s = sg2s.tile([P, H], f32, tag="g2s")
                nc.vector.tensor_copy(g2s[:, :512], g2[:, :512])
                nc.scalar.activation(g2s[:, 512:], g2[:, 512:], Copy)
                g2T = sg2T.tile([P, 8, P], f32, tag="g2T")
                for j in range(8):
                    tp = zp.tile([P, P], f32, tag="z")
                    nc.tensor.transpose(tp[:], g2s[:, j * P : (j + 1) * P], ident[:])
                    if j % 2 == 0:
                        nc.vector.tensor_copy(g2T[:, j], tp[:])
                    else:
                        nc.scalar.activation(g2T[:, j], tp[:], Copy)

                # h2 = relu(g2 @ W2 + b2), then pg3 += T.T @ h2
                h2s = sh2.tile([P, H], f32, tag="h2s")
                for half in range(2):
                    lo = half * 512
                    h2p = hp.tile([P, 512], f32, tag="h")
                    for j in range(8):
                        nc.tensor.matmul(
                            h2p[:], g2T[:, j], W2s[:, j, lo : lo + 512],
                            start=(j == 0), stop=False,
                        )
                    nc.tensor.matmul(
                        h2p[:], ones1[:1, :], b2s[:1, lo : lo + 512],
                        start=False, stop=True,
                    )
                    if half == 0:
                        nc.scalar.activation(h2s[:, lo : lo + 512], h2p[:], Relu)
                    else:
                        nc.vector.tensor_scalar_max(h2s[:, lo : lo + 512], h2p[:], 0.0)

                    cp = hp.tile([P, 512], f32, tag="h")
                    nc.tensor.matmul(
                        cp[:], Tt[:], h2s[:, lo : lo + 512], start=True, stop=True
                    )
                    nc.vector.tensor_add(
                        pg3s[:, lo : lo + 512], pg3s[:, lo : lo + 512], cp[:]
                    )

            nc.sync.dma_start(out_d[:], pg3s[:])

    nc.finalize()
    return nc


LAST_RESULT = None


def kernel(x, W1, b1, W2, b2, W3, b3, Wlin, blin, edge_index, batch, num_graphs):
    global LAST_RESULT
    from concourse.bass_utils import run_bass_kernel_spmd

    x = np.asarray(x, dtype=np.float32)
    W1 = np.asarray(W1, dtype=np.float32)
    b1 = np.asarray(b1, dtype=np.float32)
    W2 = np.asarray(W2, dtype=np.float32)
    b2 = np.asarray(b2, dtype=np.float32)
    W3 = np.asarray(W3, dtype=np.float32)
    b3 = np.asarray(b3, dtype=np.float32)
    Wlin = np.asarray(Wlin, dtype=np.float32)
    blin = np.asarray(blin, dtype=np.float32)

    agg1, aT, S, Tpad, cnt, T_w, TT = _host_prep(x, edge_index, batch)

    nc = _build_device_program(TT, T_w)

    W1b = np.zeros((4, H), dtype=np.float32)
    W1b[:2] = W1
    W1b[2] = b1
    W2r = np.ascontiguousarray(W2.reshape(8, P, H))
    b2r = b2.reshape(1, H).astype(np.float32)

    in_maps = [
        {
            "aT": np.ascontiguousarray(aT[c]),
            "S": np.ascontiguousarray(S[c].reshape(TT, P, P)),
            "T": np.ascontiguousarray(Tpad[c].reshape(NW, P, G)),
            "W1b": W1b,
            "W2": W2r,
            "b2": b2r,
        }
        for c in range(N_CORES)
    ]
    res = run_bass_kernel_spmd(nc, in_maps, core_ids=list(range(N_CORES)))
    LAST_RESULT = res
    pg3 = np.zeros((G, H), dtype=np.float64)
    for r in res.results:
        pg3 += r["pg3"].astype(np.float64)
    pg3 = pg3.astype(np.float32)

    pooled = (pg3 @ W3 + cnt[:, None] * b3[None, :]) / np.maximum(cnt, 1.0)[:, None]
    out = pooled @ Wlin + blin[None, :]
    return out.astype(np.float32)



# revision 5
# speedup vs baseline: 5.2053x; 5.2053x over previous
"""GCN (3-layer, PyG-style) forward on 8 Trainium2 NeuronCores — bf16 v2.

Math restructuring (aggregation commutes with the weight matmul; pooling is
linear):
  agg1 = Anorm @ x                        # [N,2]  tiny -> host, f64
  h1   = relu(agg1 @ W1 + b1)             # computed per-EDGE on device (msg)
  g2   = Anorm @ h1                       # dense 128x128 scatter matmul/tile
  h2   = relu(g2 @ W2 + b2)               # dense matmul per 128-node window
  pg3  = T.T @ h2  (T[src,g]=sum norm)    # pooled partial, accumulated in PSUM
  out  = ((pg3 @ W3 + cnt*b3)/max(cnt,1)) @ Wlin + blin   # host, f64

Device work is all bf16 on the tensor engine (fp32 matmul costs 4 cycles/row
vs 1 for bf16), fp32 PSUM accumulation.  Nodes are assigned to the 8*98
(core,window) bins by a balanced greedy (LPT on in-degree) so nearly every
window needs the same number of 128-edge tiles.  g2 is transposed for the
h2 matmul with the XBAR DMA-transpose (no tensor-engine transposes), and the
pooled partial pg3 stays resident in PSUM across all windows.  Edge phase of
window w+1 is emitted before the h2 phase of window w so the PE never idles
on the g2 copy/transpose.
"""

import numpy as np

N_NODES = 100000
N_EDGES = 400000
G = 128
FIN = 2
H = 1024
N_CORES = 8
P = 128
NW = 98                       # windows per core (98*128 = 12544 >= 12500)
NSLOT = NW * P                # node slots per core
NBIN = N_CORES * NW
S_CH = 8                      # S tiles per DMA chunk
A_CH = 16                     # aT tiles per DMA chunk
T_CH = 7                      # T windows per DMA chunk (98 = 14*7)

LAST_RESULT = None


def _host_prep(x, edge_index, batch):
    """All O(E) index work in numpy; returns per-core device arrays."""
    import heapq

    x = np.asarray(x, dtype=np.float32)
    ei = np.asarray(edge_index).astype(np.int64)
    batch = np.asarray(batch).astype(np.int64)
    n = N_NODES

    loops = np.arange(n, dtype=np.int64)
    row = np.concatenate([ei[0], loops])
    col = np.concatenate([ei[1], loops])

    deg = np.bincount(col, minlength=n).astype(np.float64)
    dis = np.where(deg > 0, 1.0 / np.sqrt(np.maximum(deg, 1.0)), 0.0)
    norm = dis[row] * dis[col]

    # layer-1 aggregation (FIN=2) on host, f64
    agg1 = np.empty((n, FIN), dtype=np.float32)
    for f in range(FIN):
        agg1[:, f] = np.bincount(
            col, weights=norm * x[row, f].astype(np.float64), minlength=n
        ).astype(np.float32)
    norm = norm.astype(np.float32)

    # ---- balanced node -> (core, window, pos) assignment (LPT greedy) ----
    w_node = np.bincount(col, minlength=n)          # in-degree incl self-loop
    order = np.argsort(-w_node, kind="stable")
    bin_of = np.empty(n, dtype=np.int64)
    pos_of = np.empty(n, dtype=np.int64)
    counts = np.zeros(NBIN, dtype=np.int64)
    loads = np.zeros(NBIN, dtype=np.int64)
    heap = [(0, b) for b in range(NBIN)]
    heapq.heapify(heap)
    wl = w_node[order]
    for i in range(n):
        node = order[i]
        load, b = heapq.heappop(heap)
        bin_of[node] = b
        pos_of[node] = counts[b]
        counts[b] += 1
        loads[b] = load + wl[i]
        if counts[b] < P:
            heapq.heappush(heap, (loads[b], b))
    # heavy bins share a window index across cores: sort bins by load desc,
    # deal groups of 8 to consecutive windows
    brank = np.argsort(-loads, kind="stable")
    core_of_bin = np.empty(NBIN, dtype=np.int64)
    win_of_bin = np.empty(NBIN, dtype=np.int64)
    core_of_bin[brank] = np.arange(NBIN) % N_CORES
    win_of_bin[brank] = np.arange(NBIN) // N_CORES

    node_core = core_of_bin[bin_of]
    node_win = win_of_bin[bin_of]
    node_pos = pos_of

    # ---- edge partition by destination bin ----
    ec = node_core[col]
    ew = node_win[col]
    ekey = ec * NW + ew
    eorder = np.argsort(ekey, kind="stable")
    row_s, col_s, norm_s = row[eorder], col[eorder], norm[eorder]
    ec_s, ew_s = ec[eorder], ew[eorder]
    ekey_s = ekey[eorder]

    cw_counts = np.bincount(ekey_s, minlength=NBIN).reshape(N_CORES, NW)
    tiles_per_cw = (cw_counts + P - 1) // P
    T_w = tiles_per_cw.max(axis=0)                  # shared tile count per win
    base_tile = np.concatenate([[0], np.cumsum(T_w)])
    TT = int(base_tile[-1])

    cw_starts = np.concatenate([[0], np.cumsum(cw_counts.reshape(-1))])
    idx_in_cw = np.arange(len(col_s)) - cw_starts[ekey_s]
    tile_in_w = idx_in_cw // P
    pos = idx_in_cw % P
    tile_global = base_tile[ew_s] + tile_in_w

    aT = np.zeros((N_CORES, 4, TT * P), dtype=np.float32)
    S = np.zeros((N_CORES, TT * P, P), dtype=np.float32)
    slot = tile_global * P + pos
    c = ec_s
    aT[c, 0, slot] = agg1[row_s, 0]
    aT[c, 1, slot] = agg1[row_s, 1]
    aT[c, 2, slot] = 1.0
    S[c, slot, node_pos[col_s]] = norm_s

    # ---- L3 pooling matrix T: rows indexed by source-node slot ----
    gcol = batch[col]
    rowslot = node_core[row] * NSLOT + node_win[row] * P + node_pos[row]
    Tmat = np.bincount(
        rowslot * G + gcol, weights=norm.astype(np.float64),
        minlength=N_CORES * NSLOT * G,
    ).astype(np.float32).reshape(N_CORES, NW, P, G)

    cnt = np.bincount(batch, minlength=G).astype(np.float32)
    return agg1, aT, S, Tmat, cnt, T_w, TT


def _build_device_program(TT, T_w):
    import concourse.mybir as mybir
    import concourse.tile as tile
    from concourse import bacc

    f32 = mybir.dt.float32
    bf16 = mybir.dt.bfloat16
    nc = bacc.Bacc(None, target_bir_lowering=False, debug=False)

    NCH_S = (TT + S_CH - 1) // S_CH
    NCH_A = (TT + A_CH - 1) // A_CH

    aT_d = nc.dram_tensor("aT", [4, NCH_A * A_CH * P], bf16, kind="ExternalInput")
    S_d = nc.dram_tensor("S", [NCH_S, P, S_CH * P], bf16, kind="ExternalInput")
    T_d = nc.dram_tensor("T", [NW // T_CH, P, T_CH * G], bf16, kind="ExternalInput")
    W1b_d = nc.dram_tensor("W1b", [4, H], bf16, kind="ExternalInput")
    W2_d = nc.dram_tensor("W2", [P, 8 * H], bf16, kind="ExternalInput")
    b2_d = nc.dram_tensor("b2", [1, H], bf16, kind="ExternalInput")
    out_d = nc.dram_tensor("pg3", [G, H], f32, kind="ExternalOutput")

    Relu = mybir.ActivationFunctionType.Relu
    Copy = mybir.ActivationFunctionType.Copy

    with tile.TileContext(nc) as tc:
        with (
            tc.tile_pool(name="const", bufs=1) as cst,
            tc.tile_pool(name="sa", bufs=2) as sa,
            tc.tile_pool(name="sS", bufs=2) as sS,
            tc.tile_pool(name="sT", bufs=2) as sT,
            tc.tile_pool(name="smsg", bufs=3) as smsg,
            tc.tile_pool(name="sg2s", bufs=2) as sg2s,
            tc.tile_pool(name="sg2T", bufs=2) as sg2T,
            tc.tile_pool(name="sh2", bufs=2) as sh2,
            tc.tile_pool(name="zp", bufs=2, space="PSUM") as zp,
            tc.tile_pool(name="big", bufs=2, space="PSUM") as big,
            tc.tile_pool(name="pgp", bufs=1, space="PSUM") as pgp,
        ):
            W1b = cst.tile([4, H], bf16, tag="W1b")
            nc.sync.dma_start(W1b[:], W1b_d[:])
            W2s = cst.tile([P, 8, H], bf16, tag="W2s")
            nc.sync.dma_start(W2s[:], W2_d[:].rearrange("p (j f) -> p j f", j=8))
            b2s = cst.tile([1, H], bf16, tag="b2s")
            nc.sync.dma_start(b2s[:], b2_d[:])
            ones1 = cst.tile([1, P], bf16, tag="ones1")
            nc.vector.memset(ones1[:], 1.0)

            pg3 = pgp.tile([G, H], f32, tag="pg3")

            state = {"aTc": None, "Sc": None, "Ttc": None}
            pend = []  # (g2T tile, Tt AP, w)

            def edge_phase(w):
                nt = int(T_w[w])
                t0 = int(np.sum(T_w[:w]))
                if w % T_CH == 0:
                    state["Ttc"] = sT.tile([P, T_CH * G], bf16, tag="Ttc", name="Ttc")
                    nc.sync.dma_start(state["Ttc"][:], T_d[w // T_CH])
                g2 = big.tile([P, H], f32, tag="big")
                prev = None
                for t in range(nt):
                    tg = t0 + t
                    if tg % A_CH == 0:
                        state["aTc"] = sa.tile([4, A_CH * P], bf16, tag="aTc", name="aTc")
                        nc.sync.dma_start(
                            state["aTc"][:],
                            aT_d[:, tg * P : (tg + A_CH) * P],
                        )
                    if tg % S_CH == 0:
                        state["Sc"] = sS.tile([P, S_CH * P], bf16, tag="Sc", name="Sc")
                        nc.sync.dma_start(state["Sc"][:], S_d[tg // S_CH])
                    aoff = (tg % A_CH) * P
                    soff = (tg % S_CH) * P
                    z0 = zp.tile([P, 512], f32, tag="z")
                    z1 = zp.tile([P, 512], f32, tag="z")
                    lhs_a = state["aTc"][:, aoff : aoff + P]
                    nc.tensor.matmul(z0[:], lhs_a, W1b[:, :512], start=True, stop=True)
                    nc.tensor.matmul(z1[:], lhs_a, W1b[:, 512:], start=True, stop=True)
                    msg = smsg.tile([P, H], bf16, tag="msg")
                    nc.vector.tensor_scalar_max(msg[:, :512], z0[:], 0.0)
                    nc.scalar.activation(msg[:, 512:], z1[:], Relu)
                    if prev is not None:
                        _scatter(g2, *prev, nt)
                    prev = (state["Sc"], soff, msg, t)
                _scatter(g2, *prev, nt)
                # evacuate + transpose for the h2 phase
                g2s = sg2s.tile([P, H], bf16, tag="g2s")
                nc.vector.tensor_copy(g2s[:, :512], g2[:, :512])
                nc.scalar.activation(g2s[:, 512:], g2[:, 512:], Copy)
                g2T = sg2T.tile([P, 8, P], bf16, tag="g2T")
                nc.scalar.dma_start_transpose(g2T[:], g2s[:])
                pend.append((g2T, state["Ttc"], w))

            def _scatter(g2, Sc, soff, msg, t, nt):
                lhs_s = Sc[:, soff : soff + P]
                nc.tensor.matmul(
                    g2[:, :512], lhs_s, msg[:, :512],
                    start=(t == 0), stop=(t == nt - 1),
                )
                nc.tensor.matmul(
                    g2[:, 512:], lhs_s, msg[:, 512:],
                    start=(t == 0), stop=(t == nt - 1),
                )

            def h2_pool_phase():
                g2T, Ttc, w = pend.pop(0)
                toff = (w % T_CH) * G
                h2p = big.tile([P, H], f32, tag="big")
                h2s = sh2.tile([P, H], bf16, tag="h2s")
                for half in range(2):
                    lo = half * 512
                    for j in range(8):
                        nc.tensor.matmul(
                            h2p[:, lo : lo + 512], g2T[:, j], W2s[:, j, lo : lo + 512],
                            start=(j == 0), stop=False,
                        )
                    nc.tensor.matmul(
                        h2p[:, lo : lo + 512], ones1[:1, :], b2s[:1, lo : lo + 512],
                        start=False, stop=True,
                    )
                    if half == 0:
                        nc.vector.tensor_scalar_max(h2s[:, :512], h2p[:, :512], 0.0)
                    else:
                        nc.scalar.activation(h2s[:, 512:], h2p[:, 512:], Relu)
                for half in range(2):
                    lo = half * 512
                    nc.tensor.matmul(
                        pg3[:, lo : lo + 512],
                        Ttc[:, toff : toff + G],
                        h2s[:, lo : lo + 512],
                        start=(w == 0), stop=(w == NW - 1),
                    )

            for w in range(NW):
                edge_phase(w)
                if w > 0:
                    h2_pool_phase()
            h2_pool_phase()

            pg3s = cst.tile([G, H], f32, tag="pg3s")
            nc.vector.tensor_copy(pg3s[:, :512], pg3[:, :512])
            nc.scalar.activation(pg3s[:, 512:], pg3[:, 512:], Copy)
            nc.sync.dma_start(out_d[:], pg3s[:])

    nc.finalize()
    return nc


def kernel(x, W1, b1, W2, b2, W3, b3, Wlin, blin, edge_index, batch, num_graphs):
    global LAST_RESULT
    import ml_dtypes
    from concourse.bass_utils import run_bass_kernel_spmd

    bf = ml_dtypes.bfloat16
    x = np.asarray(x, dtype=np.float32)
    W1 = np.asarray(W1, dtype=np.float32)
    b1 = np.asarray(b1, dtype=np.float32)
    W2 = np.asarray(W2, dtype=np.float32)
    b2 = np.asarray(b2, dtype=np.float32)
    W3 = np.asarray(W3, dtype=np.float32)
    b3 = np.asarray(b3, dtype=np.float32)
    Wlin = np.asarray(Wlin, dtype=np.float32)
    blin = np.asarray(blin, dtype=np.float32)

    agg1, aT, S, Tmat, cnt, T_w, TT = _host_prep(x, edge_index, batch)

    nc = _build_device_program(TT, T_w)

    NCH_S = (TT + S_CH - 1) // S_CH
    NCH_A = (TT + A_CH - 1) // A_CH

    W1b = np.zeros((4, H), dtype=np.float32)
    W1b[:2] = W1
    W1b[2] = b1
    W1b = W1b.astype(bf)
    # W2s[p, j, f] = W2[j*128+p, f]  (matches the XBAR transpose layout)
    W2r = np.ascontiguousarray(
        W2.reshape(8, P, H).transpose(1, 0, 2).reshape(P, 8 * H)
    ).astype(bf)
    b2r = b2.reshape(1, H).astype(bf)

    in_maps = []
    for c in range(N_CORES):
        # S chunks: [NCH_S, 128, S_CH*128], chunk ch row p holds tiles
        # 8ch..8ch+7 side by side
        Sc = np.zeros((NCH_S * S_CH, P, P), dtype=np.float32)
        Sc[:TT] = S[c].reshape(TT, P, P)
        Sc = np.ascontiguousarray(
            Sc.reshape(NCH_S, S_CH, P, P).transpose(0, 2, 1, 3)
            .reshape(NCH_S, P, S_CH * P)
        ).astype(bf)
        aTc = np.zeros((4, NCH_A * A_CH * P), dtype=np.float32)
        aTc[:, : TT * P] = aT[c]
        aTc = aTc.astype(bf)
        Tc = np.ascontiguousarray(
            Tmat[c].reshape(NW // T_CH, T_CH, P, G).transpose(0, 2, 1, 3)
            .reshape(NW // T_CH, P, T_CH * G)
        ).astype(bf)
        in_maps.append(
            {"aT": aTc, "S": Sc, "T": Tc, "W1b": W1b, "W2": W2r, "b2": b2r}
        )

    res = run_bass_kernel_spmd(nc, in_maps, core_ids=list(range(N_CORES)))
    LAST_RESULT = res
    pg3 = np.zeros((G, H), dtype=np.float64)
    for r in res.results:
        pg3 += r["pg3"].astype(np.float64)
    pg3 = pg3.astype(np.float32)

    pooled = (pg3 @ W3 + cnt[:, None] * b3[None, :]) / np.maximum(cnt, 1.0)[:, None]
    out = pooled @ Wlin + blin[None, :]
    return out.astype(np.float32)
